# revision 1
# baseline (speedup 1.0000x reference)
"""Multi-head GAT layer for Trainium2 — 8 heads sharded across 8 NeuronCores.

Per head h (N=4096 nodes, F=64 features):
    ltg   = graph @ W[h]                          [N, F]
    s     = ltg @ a_src,  d = ltg @ a_dst         [N]
    E     = leaky_relu(s[:, None] + d[None, :], 0.2)
    Alpha = softmax(E, axis=-1)
    out   = Alpha @ ltg

Key algebraic trick used on-device: with z = s_i + d_j and
M_ij = [z >= 0],

    exp(leaky_relu(z)) = M_ij * e^{s_i} e^{d_j} + (1-M_ij) * e^{0.2 s_i} e^{0.2 d_j}

so the whole N x N softmax reduces to ONE 0/1 mask-generation pass
(DVE tensor_scalar is_ge) plus masked matmuls on the PE:

    num_i = u_i * (M @ (v .* ltg))_i + u2_i * (T2 - (M @ (v2 .* ltg)))_i
    den_i = u_i * (M @ v)_i          + u2_i * (t2 - (M @ v2)_i)
    out_i = num_i / den_i

with u = e^s, v = e^d, u2 = e^{0.2 s}, v2 = e^{0.2 d}, and T2/t2 the
full column sums of v2 .* [ltg | 1] (complement of the mask is handled
via total-minus-masked, using identical bf16 summands for exact
consistency).  The (1-M) path never materializes.

Heads are fully independent: core h computes head h; no collectives.
"""

import os
from contextlib import ExitStack

import numpy as np

N, F_IN, F, H = 4096, 64, 64, 8
P = 128
NB = N // P           # 32 node blocks
ISUP = 4              # i-blocks per PSUM super-block (4 banks of accumulators)
NSUP = NB // ISUP     # 8 super iterations
USE_LO = bool(int(os.environ.get("GAT_USE_LO", "0")))  # hi+lo bf16 split of rhs
RC = 260 if USE_LO else 130  # R columns per j-block
# fraction of mask-generation work routed to ScalarE as sigmoid(BIG*z)
# (saturates to exact 0/1 away from the kink; kink itself is continuous)
SIG_NUM = int(os.environ.get("GAT_SIG_NUM", "0"))
SIG_DEN = int(os.environ.get("GAT_SIG_DEN", "16"))
SIG_SCALE = 65536.0
_CACHE = {}


def _build():
    import concourse.bass as bass  # noqa: F401
    import concourse.mybir as mybir
    import concourse.tile as tile
    from concourse import bacc

    dt = mybir.dt
    f32 = dt.float32
    bf16 = dt.bfloat16
    Alu = mybir.AluOpType
    Act = mybir.ActivationFunctionType

    nc = bacc.Bacc("TRN2", debug=False, num_devices=H)
    graph_d = nc.dram_tensor("graph", [N, F_IN], f32, kind="ExternalInput").ap()
    w_d = nc.dram_tensor("w", [F_IN, F], f32, kind="ExternalInput").ap()
    a_d = nc.dram_tensor("a", [2, F], f32, kind="ExternalInput").ap()
    out_d = nc.dram_tensor("out", [N, F], f32, kind="ExternalOutput").ap()

    ident_d = nc.inline_tensor(np.eye(P, dtype=np.float32), name="ident")

    with tile.TileContext(nc) as tc, ExitStack() as ctx:
        persist = ctx.enter_context(tc.tile_pool(name="persist", bufs=1))
        # setup PSUM (tp x2 + pj x2 = 4 banks) coexists with acc (4 banks)
        sps = ctx.enter_context(tc.tile_pool(name="sps", bufs=2, space="PSUM"))
        accp = ctx.enter_context(tc.tile_pool(name="acc", bufs=1, space="PSUM"))
        ssb = ctx.enter_context(tc.tile_pool(name="ssb", bufs=4))
        gp = ctx.enter_context(tc.tile_pool(name="gp", bufs=6))
        mp = ctx.enter_context(tc.tile_pool(name="mask", bufs=3))
        ep = ctx.enter_context(tc.tile_pool(name="ep", bufs=6))

        identity = persist.tile([P, P], f32)
        nc.sync.dma_start(identity[:], ident_d.ap())
        ones_row = persist.tile([1, P], f32)
        nc.vector.memset(ones_row[:], 1.0)
        ones_row_bf = persist.tile([1, P], bf16)
        nc.vector.memset(ones_row_bf[:], 1.0)
        ones_col_bf = persist.tile([P, 1], bf16)
        nc.vector.memset(ones_col_bf[:], 1.0)

        # fused [W | w_s | w_d] rhs for the per-block projection matmul
        wssd = persist.tile([F_IN, F + 2], f32)
        nc.sync.dma_start(wssd[:, 0:F], w_d[:])
        a2_sb = persist.tile([F, 2], f32)
        nc.sync.dma_start(a2_sb[:], a_d.rearrange("t k -> k t"))

        gT = persist.tile([F_IN, N], f32)            # graph^T
        ltgsd = persist.tile([P, 66 * NB], f32)      # per b: ltg (64) | s | d
        negsd = persist.tile([P, 2 * NB], f32)       # -s, -d columns
        dscaled = persist.tile([P, 2 * NB], f32)     # SIG_SCALE * (s, d)
        uv1 = persist.tile([P, 2 * NB], f32)         # exp(s), exp(d)
        uv2 = persist.tile([P, 2 * NB], f32)         # exp(.2 s), exp(.2 d)
        sdrow = persist.tile([2, N], bf16)           # s, d rows (bcast feed)
        s_rep = persist.tile([P, N], bf16)           # s broadcast down partitions
        r_all = persist.tile([P, RC * NB], bf16)     # [R1|R2|v|v2] (+lo) per b
        t2acc = persist.tile([1, 66], f32)           # T2 row (SBUF copy)
        t2rep = persist.tile([P, 66], f32)           # T2 bcast down partitions
        eps_all = persist.tile([P, 130 * NB], f32)   # psum snapshots per i-block

        ltgsd_v = ltgsd.rearrange("p (b c) -> p b c", c=66)
        r_v = r_all.rearrange("p (b c) -> p b c", c=RC)
        uv1_v = uv1.rearrange("p (b c) -> p b c", c=2)
        uv2_v = uv2.rearrange("p (b c) -> p b c", c=2)
        eps_v = eps_all.rearrange("p (b c) -> p b c", c=130)

        # W^T, then [w_s | w_d] = W^T.T @ a2
        wT_ps = sps.tile([F, F_IN], f32, tag="tp")
        nc.tensor.transpose(wT_ps[:], wssd[:, 0:F], identity[0:F_IN, 0:F_IN])
        wT_sb = ssb.tile([F, F_IN], f32)
        nc.vector.tensor_copy(wT_sb[:], wT_ps[:])
        wsd_ps = sps.tile([F_IN, 2], f32, tag="pj")
        nc.tensor.matmul(wsd_ps[:], wT_sb[:], a2_sb[:])
        nc.vector.tensor_copy(wssd[:, F:F + 2], wsd_ps[:])

        mask_tiles = {}

        def emit_mask(sup, b):
            i0 = sup * ISUP * P
            mt = mp.tile([P, ISUP * P], bf16, tag=f"m{b}", name=f"mask{b}")
            if (b % SIG_DEN) < SIG_NUM:
                nc.scalar.activation(
                    mt[:], s_rep[:, i0:i0 + ISUP * P], Act.Sigmoid,
                    bias=dscaled[:, 2 * b + 1:2 * b + 2], scale=SIG_SCALE)
            else:
                nc.vector.tensor_scalar(
                    mt[:], s_rep[:, i0:i0 + ISUP * P],
                    negsd[:, 2 * b + 1:2 * b + 2], None, op0=Alu.is_ge)
            mask_tiles[(sup, b)] = mt

        def do_group(g):
            """s/d-derived tables + R blocks for blocks 4g..4g+3."""
            sd_src = ltgsd_v[:, 4 * g:4 * g + 4, F:F + 2]
            dst = slice(8 * g, 8 * g + 8)
            nc.vector.tensor_scalar(negsd[:, dst], sd_src, -1.0, None,
                                    op0=Alu.mult)
            if SIG_NUM:
                nc.vector.tensor_scalar(dscaled[:, dst], sd_src, SIG_SCALE,
                                        None, op0=Alu.mult)
            nc.scalar.activation(uv1[:, dst], sd_src, Act.Exp)
            nc.scalar.activation(uv2[:, dst], sd_src, Act.Exp, scale=0.2)
            for bb in range(4 * g, 4 * g + 4):
                ltg_b = ltgsd[:, 66 * bb:66 * bb + F]
                v_col = uv1[:, 2 * bb + 1:2 * bb + 2]
                v2_col = uv2[:, 2 * bb + 1:2 * bb + 2]
                r0 = RC * bb
                if not USE_LO:
                    nc.vector.tensor_scalar(r_all[:, r0:r0 + F], ltg_b, v_col,
                                            None, op0=Alu.mult)
                    nc.scalar.mul(r_all[:, r0 + F:r0 + 2 * F], ltg_b, v2_col)
                else:
                    r1f = ssb.tile([P, F], f32, tag="r1f", name="r1f")
                    r2f = ssb.tile([P, F], f32, tag="r2f", name="r2f")
                    nc.vector.tensor_scalar(r1f[:], ltg_b, v_col, None,
                                            op0=Alu.mult)
                    nc.vector.tensor_scalar(r2f[:], ltg_b, v2_col, None,
                                            op0=Alu.mult)
                    nc.scalar.copy(r_all[:, r0:r0 + F], r1f[:])
                    nc.scalar.copy(r_all[:, r0 + F:r0 + 2 * F], r2f[:])
                    nc.vector.tensor_tensor(r_all[:, r0 + 130:r0 + 130 + F],
                                            r1f[:], r_all[:, r0:r0 + F],
                                            op=Alu.subtract)
                    nc.vector.tensor_tensor(
                        r_all[:, r0 + 130 + F:r0 + 130 + 2 * F], r2f[:],
                        r_all[:, r0 + F:r0 + 2 * F], op=Alu.subtract)
            bsl = slice(4 * g, 4 * g + 4)
            nc.vector.tensor_copy(r_v[:, bsl, 128], uv1_v[:, bsl, 1])
            nc.vector.tensor_copy(r_v[:, bsl, 129], uv2_v[:, bsl, 1])
            if USE_LO:
                nc.vector.tensor_tensor(r_v[:, bsl, 258], uv1_v[:, bsl, 1],
                                        r_v[:, bsl, 128], op=Alu.subtract)
                nc.vector.tensor_tensor(r_v[:, bsl, 259], uv2_v[:, bsl, 1],
                                        r_v[:, bsl, 129], op=Alu.subtract)

        # pipelined setup over 32 blocks
        for b in range(NB):
            g_sb = gp.tile([P, F_IN], f32)
            nc.sync.dma_start(g_sb[:], graph_d[b * P:(b + 1) * P, :])
            gT_ps = sps.tile([F_IN, P], f32, tag="tp")
            nc.tensor.transpose(gT_ps[:], g_sb[:], identity[:])
            nc.scalar.copy(gT[:, b * P:(b + 1) * P], gT_ps[:])
            prj_ps = sps.tile([P, F + 2], f32, tag="pj")
            nc.tensor.matmul(prj_ps[:], gT[:, b * P:(b + 1) * P], wssd[:])
            nc.scalar.copy(ltgsd[:, 66 * b:66 * (b + 1)], prj_ps[:])
            if b % 4 == 3:
                c = b // 4
                srow_ps = sps.tile([2, 512], f32, tag="pj", name="srow_ps")
                nc.tensor.matmul(srow_ps[:], wssd[:, F:F + 2],
                                 gT[:, c * 512:(c + 1) * 512])
                nc.scalar.copy(sdrow[:, c * 512:(c + 1) * 512], srow_ps[:])
                bc_ps = sps.tile([P, 512], f32, tag="tp", name="bc_ps")
                nc.tensor.matmul(bc_ps[:], ones_row_bf[:],
                                 sdrow[0:1, c * 512:(c + 1) * 512])
                nc.scalar.copy(s_rep[:, c * 512:(c + 1) * 512], bc_ps[:])
            if b % 4 == 3:
                g = b // 4
                do_group(g)
                # prefill masks for the first supers (sup k needs s_rep chunk k,
                # ready after setup block 4k+3)
                for sup in range(min(3, b // 4 + 1)):
                    for bb in range(4 * g, 4 * g + 4):
                        emit_mask(sup, bb)

        # T2 burst: transient psum tile, released right after the copy
        t2_ps = sps.tile([1, 66], f32, tag="pj", name="t2ps_g")
        n_acc = NB * (2 if USE_LO else 1)
        k = 0
        for bb in range(NB):
            r0 = RC * bb
            nc.tensor.matmul(t2_ps[:], ones_col_bf[:],
                             r_all[:, r0 + F:r0 + 130],
                             start=(k == 0), stop=(k == n_acc - 1))
            k += 1
            if USE_LO:
                nc.tensor.matmul(t2_ps[:], ones_col_bf[:],
                                 r_all[:, r0 + 130 + F:r0 + 260],
                                 start=False, stop=(k == n_acc - 1))
                k += 1
        nc.vector.tensor_copy(t2acc[:], t2_ps[:])
        t2rep_ps = sps.tile([P, 66], f32, tag="tp", name="t2rep_ps")
        nc.tensor.matmul(t2rep_ps[:], ones_row[:], t2acc[:])
        nc.scalar.copy(t2rep[:], t2rep_ps[:])

        # ---- main masked-matmul loop ----

        late = []

        def late_phase(sup):
            """Epilogue arithmetic on SBUF snapshots; needs t2rep."""
            i0 = sup * ISUP
            u_v = uv1_v[:, i0:i0 + ISUP, 0]
            u2_v = uv2_v[:, i0:i0 + ISUP, 0]
            den1 = ep.tile([P, ISUP], f32, tag="den1", name="den1")
            nc.vector.tensor_tensor(den1[:], u_v, eps_v[:, i0:i0 + ISUP, 128],
                                    op=Alu.mult)
            dd = ep.tile([P, ISUP], f32, tag="dd", name="dd")
            nc.vector.tensor_tensor(dd[:], t2rep[:, 65:66].to_broadcast([P, ISUP]),
                                    eps_v[:, i0:i0 + ISUP, 129], op=Alu.subtract)
            den2 = ep.tile([P, ISUP], f32, tag="den2", name="den2")
            nc.vector.tensor_tensor(den2[:], dd[:], u2_v, op=Alu.mult)
            den = ep.tile([P, ISUP], f32, tag="den", name="den")
            nc.vector.tensor_tensor(den[:], den2[:], den1[:], op=Alu.add)
            rden = ep.tile([P, ISUP], f32, tag="rden", name="rden")
            nc.vector.reciprocal(rden[:], den[:])
            for t in range(ISUP):
                i = i0 + t
                e0 = 130 * i
                u_col = uv1[:, 2 * i:2 * i + 1]
                u2_col = uv2[:, 2 * i:2 * i + 1]
                n1 = ep.tile([P, F], f32, tag="n1", name="n1")
                nc.scalar.mul(n1[:], eps_all[:, e0:e0 + F], u_col)
                d2 = ep.tile([P, F], f32, tag="d2", name="d2")
                nc.vector.tensor_tensor(d2[:], t2rep[:, 0:F],
                                        eps_all[:, e0 + F:e0 + 2 * F],
                                        op=Alu.subtract)
                n2 = ep.tile([P, F], f32, tag="n2", name="n2")
                nc.scalar.mul(n2[:], d2[:], u2_col)
                num = ep.tile([P, F], f32, tag="num", name="num")
                nc.vector.tensor_tensor(num[:], n1[:], n2[:], op=Alu.add)
                ot = ep.tile([P, F], f32, tag="ot", name="ot")
                nc.scalar.mul(ot[:], num[:], rden[:, t:t + 1])
                nc.sync.dma_start(out_d[i * P:(i + 1) * P, :], ot[:])

        for sup in range(NSUP):
            for b in range(NB):
                if (sup, b) not in mask_tiles:
                    emit_mask(sup, b)
            mtiles = [mask_tiles.pop((sup, b)) for b in range(NB)]

            acc = accp.tile([P, 512 * ISUP], f32, name="acc")
            for b in range(NB):
                r0 = RC * b
                for t in range(ISUP):
                    nc.tensor.matmul(
                        acc[:, 512 * t:512 * t + RC],
                        mtiles[b][:, t * P:(t + 1) * P],
                        r_all[:, r0:r0 + RC],
                        start=(b == 0), stop=(b == NB - 1))
            # snapshot psum -> SBUF (releases the accumulator quickly)
            for t in range(ISUP):
                i = sup * ISUP + t
                if USE_LO:
                    lo_sb = ep.tile([P, 130], f32, tag="lo_sb", name="lo_sb")
                    nc.scalar.copy(lo_sb[:],
                                   acc[:, 512 * t + 130:512 * t + 260])
                    nc.vector.tensor_tensor(eps_all[:, 130 * i:130 * (i + 1)],
                                            acc[:, 512 * t:512 * t + 130],
                                            lo_sb[:], op=Alu.add)
                else:
                    nc.scalar.copy(eps_all[:, 130 * i:130 * (i + 1)],
                                   acc[:, 512 * t:512 * t + 130])
            late.append(sup)
            if len(late) > 1:
                late_phase(late.pop(0))
        for sup in late:
            late_phase(sup)

    nc.compile()
    return nc


def _get_nc():
    if "nc" not in _CACHE:
        _CACHE["nc"] = _build()
    return _CACHE["nc"]


def kernel(graph, W, a):
    from concourse.bass_utils import run_bass_kernel_spmd

    graph = np.ascontiguousarray(np.asarray(graph, dtype=np.float32))
    W = np.asarray(W, dtype=np.float32)
    a = np.asarray(a, dtype=np.float32)

    nc = _get_nc()
    in_maps = [
        {
            "graph": graph,
            "w": np.ascontiguousarray(W[h]),
            "a": np.ascontiguousarray(a[h].reshape(2, F)),
        }
        for h in range(H)
    ]
    trace = bool(int(os.environ.get("GAT_TRACE", "0")))
    res = run_bass_kernel_spmd(nc, in_maps, core_ids=list(range(H)), trace=trace)
    _CACHE["last_result"] = res
    return np.stack([res.results[h]["out"] for h in range(H)], axis=0)

